# revision 2
# baseline (speedup 1.0000x reference)
"""Bahdanau-attention TRN2 Bass kernel — nn_BahdanauAttention_69106023793233.

Strategy (B=32 sharded over 8 NeuronCores, 4 batches/core, single HBM pass):
  phase 1 (per core): value[bl] --SWDGE cast-dma--> V16 (bf16, SBUF-resident)
      V16 --PE transpose (bf16 identity)--> VT tiles [d,s] (PSUM->SBUF copy)
      w2kT = W2.T @ VT  (bf16 matmul, fp32 PSUM)
      T = tanh(w2kT + w1qT[:,bl])  (ACT, per-partition bias)  -> bf16
      scores chunk += Wo4(bl).T @ T  (masked-column stationary packs 4 batch
      rows into one [4,512] PSUM bank);  scores_sb = chunk + bo
  softmax: exp with accumulated row-sums (ACT accum_out), normalize -> bf16
  AllGather attn (64KB) across the 8 cores
  phase 2: aw[bl][c=32, dv=512] += attnT(t).T @ V16[bl,t] over 32 s-tiles,
      then PE-transpose to [dv, c] and DMA out.
Host side only shards/replicates inputs and concatenates per-core outputs.
"""
import numpy as np

import concourse.bacc as bacc
import concourse.bass as bass
import concourse.mybir as mybir
import concourse.tile as tile
import concourse.masks as masks
from concourse.bass_utils import run_bass_kernel_spmd

F32 = mybir.dt.float32
BF16 = mybir.dt.bfloat16
AF = mybir.ActivationFunctionType

N_CORES = 8
B, S, D, U = 32, 4096, 512, 256
NB = B // N_CORES          # 4 local batches per core
NT = S // 128              # 32 s-tiles per batch
NCH = S // 512             # 8 s-chunks per batch

_CACHE = {}


def _build():
    nc = bacc.Bacc("TRN2", target_bir_lowering=False, debug=False,
                   enable_asserts=False, num_devices=N_CORES)

    value_l = nc.dram_tensor("value_l", [NB, S, D], F32, kind="ExternalInput")
    queryT_l = nc.dram_tensor("queryT_l", [D, NB], F32, kind="ExternalInput")
    W1_t = nc.dram_tensor("W1", [D, U], F32, kind="ExternalInput")
    W2_t = nc.dram_tensor("W2", [D, U], F32, kind="ExternalInput")
    Wo_t = nc.dram_tensor("Wo", [U, 1], F32, kind="ExternalInput")
    bo_t = nc.dram_tensor("bo4", [NB, 1], F32, kind="ExternalInput")

    scores_out = nc.dram_tensor("scores_out", [NB, S], F32, kind="ExternalOutput")
    aw_out = nc.dram_tensor("aw_out", [NB, D, B], F32, kind="ExternalOutput")

    with tile.TileContext(nc) as tc:
        with (
            tc.tile_pool(name="res", bufs=1) as res,
            tc.tile_pool(name="work", bufs=2) as work,
            tc.tile_pool(name="dram", bufs=1, space="DRAM") as dram,
        ):
            # ------------- constants -------------
            W2_16 = res.tile([128, 4 * U], BF16)
            for i in range(4):
                nc.gpsimd.dma_start(out=W2_16[:, i * U:(i + 1) * U],
                                    in_=W2_t[128 * i:128 * (i + 1), :])
            Wo16 = res.tile([128, 2], BF16)
            for j in range(2):
                nc.gpsimd.dma_start(out=Wo16[:, j:j + 1],
                                    in_=Wo_t[128 * j:128 * (j + 1), :])
            W1sb = res.tile([128, 4 * U], F32)
            for i in range(4):
                nc.scalar.dma_start(out=W1sb[:, i * U:(i + 1) * U],
                                    in_=W1_t[128 * i:128 * (i + 1), :])
            qT = res.tile([128, 4 * NB], F32)
            for i in range(4):
                nc.scalar.dma_start(out=qT[:, i * NB:(i + 1) * NB],
                                    in_=queryT_l[128 * i:128 * (i + 1), :])
            bo_sb = res.tile([NB, 1], F32)
            nc.scalar.dma_start(out=bo_sb[:], in_=bo_t[:])
            ident = res.tile([128, 128], F32)
            masks.make_identity(nc, ident[:])
            ident16 = res.tile([128, 128], BF16)
            masks.make_identity(nc, ident16[:])

            # Wo4: 8 stationaries [128,4]; column bl of block (j,bl) holds
            # Wo[u-half j], the rest are zero -> one matmul per (j,bl) adds
            # only row bl of the [4,512] scores PSUM tile.
            Wo4 = res.tile([128, 32], BF16)
            nc.gpsimd.memset(Wo4[:], 0.0)
            for j in range(2):
                for bl in range(NB):
                    col = j * 16 + 4 * bl + bl
                    nc.vector.tensor_copy(Wo4[:, col:col + 1], Wo16[:, j:j + 1])

            V16 = res.tile([128, NB * NT * D], BF16)
            scores_sb = res.tile([NB, S], F32)

            # ------------- phase 1 -------------
            with tc.tile_pool(name="psA", bufs=2, space="PSUM") as psA:
                psq = psA.tile([128, 2 * NB], F32, tag="psc", name="psq", bufs=1)
                for j in range(2):
                    for i in range(4):
                        nc.tensor.matmul(psq[:, NB * j:NB * (j + 1)],
                                         W1sb[:, U * i + 128 * j:U * i + 128 * j + 128],
                                         qT[:, NB * i:NB * (i + 1)],
                                         start=(i == 0), stop=(i == 3))
                w1q_sb = res.tile([128, 2 * NB], F32)
                nc.vector.tensor_copy(w1q_sb[:], psq[:])

                for k in range(NCH):
                    psc = psA.tile([NB, 512], F32, tag="psc", name="psc", bufs=1)
                    for bl in range(NB):
                        base = (bl * NT + 4 * k) * D
                        nc.gpsimd.dma_start(
                            out=V16[:, base:base + 4 * D].rearrange("p (t d) -> p t d", d=D),
                            in_=value_l[bl, 512 * k:512 * (k + 1), :].rearrange("(t p) d -> p t d", p=128),
                        )
                        vts = []
                        for i in range(4):
                            ptr = psA.tile([128, 512], BF16, tag="psT", name="psT", bufs=3)
                            for t in range(4):
                                nc.tensor.transpose(
                                    ptr[:, 128 * t:128 * (t + 1)],
                                    V16[:, base + t * D + 128 * i: base + t * D + 128 * (i + 1)],
                                    ident16[:],
                                )
                            vt = work.tile([128, 512], BF16, tag=f"vt{i}",
                                           name=f"vt{i}", bufs=3)
                            if i % 2 == 0:
                                nc.vector.tensor_copy(vt[:], ptr[:])
                            else:
                                nc.scalar.activation(vt[:], ptr[:], AF.Copy)
                            vts.append(vt)
                        for j in range(2):
                            pw = psA.tile([128, 512], F32, tag=f"pw{j}",
                                          name=f"pw{j}", bufs=2)
                            for i in range(4):
                                nc.tensor.matmul(pw[:],
                                                 W2_16[:, U * i + 128 * j:U * i + 128 * j + 128],
                                                 vts[i][:], start=(i == 0), stop=(i == 3))
                            t16 = work.tile([128, 512], BF16, tag=f"t16{j}",
                                            name=f"t16{j}", bufs=3)
                            nc.scalar.activation(t16[:], pw[:], AF.Tanh,
                                                 bias=w1q_sb[:, NB * j + bl:NB * j + bl + 1])
                            nc.tensor.matmul(psc[:],
                                             Wo4[:, j * 16 + 4 * bl:j * 16 + 4 * bl + 4],
                                             t16[:], start=(j == 0 and bl == 0),
                                             stop=(j == 1 and bl == NB - 1))
                    nc.scalar.activation(scores_sb[:, 512 * k:512 * (k + 1)], psc[:],
                                         AF.Identity, bias=bo_sb[:])

            nc.scalar.dma_start(out=scores_out[:], in_=scores_sb[:])

            # ------------- softmax + gather -------------
            attn16 = res.tile([NB, S], BF16)
            sums = res.tile([NB, 1], F32)
            nc.scalar.activation(attn16[:], scores_sb[:], AF.Exp, accum_out=sums[:])
            recip = res.tile([NB, 1], F32)
            nc.vector.reciprocal(recip[:], sums[:])
            nc.vector.tensor_scalar_mul(attn16[:], attn16[:], recip[:])

            attn_in = dram.tile([NB, S], BF16)
            nc.scalar.dma_start(out=attn_in[:], in_=attn16[:])
            attn_out = dram.tile([B, S], BF16)
            nc.gpsimd.collective_compute(
                "AllGather", mybir.AluOpType.bypass,
                replica_groups=[list(range(N_CORES))],
                ins=[attn_in.opt()], outs=[attn_out.opt()],
            )
            attn_all = res.tile([B, S], BF16)
            nc.scalar.dma_start(out=attn_all[:], in_=attn_out[:])

            attnT = res.tile([128, B * NT], BF16)

            # ------------- phase 2 -------------
            with tc.tile_pool(name="psB", bufs=1, space="PSUM") as psB:
                for t in range(NT):
                    pat = psB.tile([128, B], BF16, tag="pt", name="pat", bufs=2)
                    nc.tensor.transpose(pat[:], attn_all[0:B, 128 * t:128 * (t + 1)],
                                        ident16[0:B, 0:B])
                    nc.vector.tensor_copy(attnT[:, B * t:B * (t + 1)], pat[:])

                paws = [psB.tile([B, 512], F32, tag=f"paw{bl}", name=f"paw{bl}")
                        for bl in range(NB)]
                for t in range(NT):
                    for bl in range(NB):
                        nc.tensor.matmul(paws[bl][:],
                                         attnT[:, B * t:B * (t + 1)],
                                         V16[:, (bl * NT + t) * D:(bl * NT + t + 1) * D],
                                         start=(t == 0), stop=(t == NT - 1))
                for bl in range(NB):
                    aw_sb = work.tile([B, 512], F32, tag="aw_sb", name="aw_sb")
                    nc.vector.tensor_copy(aw_sb[:], paws[bl][:])
                    for i in range(4):
                        pt = psB.tile([128, B], F32, tag="pt", name="pt", bufs=2)
                        nc.tensor.transpose(pt[:], aw_sb[:, 128 * i:128 * (i + 1)],
                                            ident[0:B, 0:B])
                        awT = work.tile([128, B], F32, tag="awT", name="awT")
                        nc.vector.tensor_copy(awT[:], pt[:])
                        nc.scalar.dma_start(out=aw_out[bl, 128 * i:128 * (i + 1), :],
                                            in_=awT[:])

    nc.compile()
    return nc


def kernel(query, value, target_mask, W1, W2, Wo, bo):
    # target_mask is unused by the reference math (all-ones mask).
    if "nc" not in _CACHE:
        _CACHE["nc"] = _build()
    nc = _CACHE["nc"]

    value = np.ascontiguousarray(np.asarray(value, dtype=np.float32))
    query = np.asarray(query, dtype=np.float32)
    W1 = np.ascontiguousarray(np.asarray(W1, dtype=np.float32))
    W2 = np.ascontiguousarray(np.asarray(W2, dtype=np.float32))
    Wo = np.ascontiguousarray(np.asarray(Wo, dtype=np.float32)).reshape(U, 1)
    bo = np.asarray(bo, dtype=np.float32)

    in_maps = []
    for c in range(N_CORES):
        sl = slice(c * NB, (c + 1) * NB)
        in_maps.append({
            "value_l": np.ascontiguousarray(value[sl]),
            "queryT_l": np.ascontiguousarray(query[sl].T),
            "W1": W1, "W2": W2, "Wo": Wo,
            "bo4": np.full((NB, 1), float(bo[0]), np.float32),
        })

    results = run_bass_kernel_spmd(nc, in_maps, list(range(N_CORES))).results
    scores = np.concatenate([r["scores_out"] for r in results], axis=0)
    aw = np.concatenate([r["aw_out"] for r in results], axis=0)
    return scores, aw
